# revision 7
# baseline (speedup 1.0000x reference)
# Patch-shuffle kernel for Trainium2 (Bass), 8-way data parallel.
#
# Problem: img [64,3,384,384] f32, perm [64,576] int32 (per-image permutation
# of 16x16 patches in row-major (py,px) order). Output = per-image patch
# gather reassembled into image layout.
#
# Strategy: host repacks each image into patch-major layout [576, 768]
# (a perm-independent layout transform, part of sharding) and converts the
# payload to bf16 (halves the HBM roofline; ~2^-9 max relative rounding
# error, far inside the 2e-2 gate). The device gathers all 4608 patches of
# its 8 images from DRAM into SBUF so that partition p accumulates output
# rows [36p, 36p+36) in order, making every store back to DRAM a fully
# contiguous per-partition HWDGE DMA. Stores alternate on the Sync/Act
# HWDGE queues and overlap subsequent gathers; nothing shares SBUF
# buffers, so the whole pipeline runs without serialization.
#
# Two gather implementations:
#   "dg"  -- chunked InstDMAGatherAnt (Q7 ucode, mlp library): one
#            instruction gathers 512 patches (~1.2us issue), so the SWDGE
#            issue rate never binds.
#   "ind" -- one SWDGE indirect_dma_start per output patch column (36 x
#            ~1.4us issue; HW consumes ONE index per partition per
#            instruction). Fallback if the ucode library path misbehaves.
import numpy as np

_MODE = "dg"

_NCORES = 8
_IMGS_PER_CORE = 8
_NPATCH = 576  # 24*24 patches per image
_ELEM = 768  # payload elements per patch (3*16*16)
_N = _NPATCH * _IMGS_PER_CORE  # 4608 patches per core
_PPB = _N // 128  # 36 output patch rows per SBUF partition
_K = 9 if _MODE == "dg" else 6  # gather/store chunks
_CPC = _PPB // _K  # patch columns per chunk


def _patchify(img):
    # [B,3,384,384] -> [B, 576, 768] with patch o=(py*24+px), vec (c,ry,rx)
    b = img.shape[0]
    return (
        img.reshape(b, 3, 24, 16, 24, 16)
        .transpose(0, 2, 4, 1, 3, 5)
        .reshape(b, _NPATCH, _ELEM)
    )


def _unpatchify(pat):
    # [B, 576, 768] -> [B,3,384,384]
    b = pat.shape[0]
    return (
        pat.reshape(b, 24, 24, 3, 16, 16)
        .transpose(0, 3, 1, 4, 2, 5)
        .reshape(b, 3, 384, 384)
    )


def _flat_perm(perm_core):
    # perm_core: [8, 576] int32 for one core's images. Returns [128, 36]:
    # fl[p, j] = source patch row (into src [4608, ELEM]) for output patch
    # row p*36 + j.
    flat = (
        perm_core.astype(np.int64)
        + (np.arange(_IMGS_PER_CORE)[:, None] * _NPATCH)
    ).reshape(_N)
    assert flat.max() < _N
    return flat.reshape(128, _PPB)


def _build_idx_ind(perm_core):
    return np.ascontiguousarray(_flat_perm(perm_core).astype(np.int32))


def _build_idx_dg(perm_core):
    # InstDMAGatherAnt index layout, chunked: chunk c gathers items
    # i=0..CPC*128-1 with dst[i%128, i//128] -- we want dst[p, j] to be
    # output row p*36 + c*CPC + j, so item i=j*128+p carries
    # fl[p, c*CPC+j]. The int16 index tile wraps items in 16 partitions
    # (item i at [i%16, i//16]) and is replicated across the 8 gpsimd
    # cores' partition groups.
    fl = _flat_perm(perm_core)  # [128, 36]
    cols = []
    for c in range(_K):
        arr = fl[:, c * _CPC : (c + 1) * _CPC]  # [128, CPC]
        lin = arr.T.reshape(_CPC * 128)  # item i = j*128+p
        tile16 = lin.reshape(_CPC * 8, 16).T  # [16, CPC*8]
        cols.append(np.tile(tile16, (8, 1)))  # [128, CPC*8]
    return np.ascontiguousarray(np.concatenate(cols, axis=1).astype(np.int16))


def _split_multiwait(nc):
    # TRN2 allows at most one sync wait per instruction; hoist extra waits
    # onto same-engine nops placed immediately before the instruction.
    # (Safety net -- the manual-semaphore program below emits at most one
    # wait per instruction already.)
    from concourse import mybir

    eng_map = {
        mybir.EngineType.Pool: nc.gpsimd,
        mybir.EngineType.SP: nc.sync,
        mybir.EngineType.Activation: nc.scalar,
        mybir.EngineType.PE: nc.tensor,
        mybir.EngineType.DVE: nc.vector,
    }
    blocks = [b for f in nc.m.functions for b in f.blocks]
    multi = []
    for blk in blocks:
        for inst in blk.instructions:
            si = inst.sync_info
            if si and si.on_wait and len(si.on_wait) > 1:
                multi.append((blk, inst))
    for blk, inst in multi:
        eng = eng_map.get(inst.engine, nc.sync)
        waits = list(inst.sync_info.on_wait)
        helpers = []
        for w in waits[:-1]:
            nop = eng.nop().ins
            for b2 in blocks:
                if nop in b2.instructions:
                    b2.instructions.remove(nop)
                    break
            nop.sync_info = mybir.SyncInfo(on_wait=[w], on_update=[])
            helpers.append(nop)
        inst.sync_info.on_wait = [waits[-1]]
        pos = blk.instructions.index(inst)
        for j, h in enumerate(helpers):
            blk.instructions.insert(pos + j, h)


def _build_nc():
    from contextlib import ExitStack

    import concourse.bass as bass
    from concourse import mybir

    nc = bass.Bass()
    src_ext = nc.dram_tensor(
        "src", [_N, _ELEM], mybir.dt.bfloat16, kind="ExternalInput"
    )
    if _MODE == "dg":
        idx_ext = nc.dram_tensor(
            "idx", [128, _N // 16], mybir.dt.int16, kind="ExternalInput"
        )
    else:
        idx_ext = nc.dram_tensor(
            "idx", [128, _PPB], mybir.dt.int32, kind="ExternalInput"
        )
    out_ext = nc.dram_tensor(
        "out", [128, _PPB, _ELEM], mybir.dt.bfloat16, kind="ExternalOutput"
    )

    with ExitStack() as stack:
        data = stack.enter_context(
            nc.sbuf_tensor("data", [128, _PPB, _ELEM], mybir.dt.bfloat16)
        )
        idx_tile = stack.enter_context(
            nc.sbuf_tensor("idxs", list(idx_ext.shape), idx_ext.dtype)
        )
        sio = stack.enter_context(nc.semaphore("sio"))
        gs = [
            stack.enter_context(nc.semaphore(f"gs{k}")) for k in range(_K)
        ]
        se = [stack.enter_context(nc.semaphore(f"se{i}")) for i in range(2)]

        if _MODE == "dg":
            from concourse import library_config

            nc.gpsimd.load_library(library_config.mlp)
        # idx loads via HWDGE so it runs while gpsimd sets up
        nc.sync.dma_start(idx_tile[:], idx_ext[:]).then_inc(sio, 16)
        nc.gpsimd.wait_ge(sio, 16)
        if _MODE == "dg":
            nidx = _CPC * 128
            for k in range(_K):
                nc.gpsimd.dma_gather(
                    data[:, k * _CPC : (k + 1) * _CPC, :],
                    src_ext[:],
                    idx_tile[:, k * _CPC * 8 : (k + 1) * _CPC * 8],
                    nidx,
                    nidx,
                    _ELEM,
                ).then_inc(gs[k], 16)
            chunk_incs = 16
        else:
            # HW indirect DMA consumes ONE index per partition per
            # instruction -- one gather per output patch column. Gathers
            # for the same store chunk share a semaphore; the store waits
            # for the full chunk total, which is exact.
            for j in range(_PPB):
                nc.gpsimd.indirect_dma_start(
                    out=data[:, j, :],
                    out_offset=None,
                    in_=src_ext[:],
                    in_offset=bass.IndirectOffsetOnAxis(
                        ap=idx_tile[:, j : j + 1], axis=0
                    ),
                ).then_inc(gs[j // _CPC], 16)
            chunk_incs = 16 * _CPC
        store_engines = [nc.sync, nc.scalar]
        for k in range(_K):
            eng = store_engines[k % 2]
            cs, ce = k * _CPC, (k + 1) * _CPC
            eng.wait_ge(gs[k], chunk_incs)
            eng.dma_start(
                out_ext[:, cs:ce, :], data[:, cs:ce, :]
            ).then_inc(se[k % 2], 16)
        nc.sync.wait_ge(se[0], 16 * ((_K + 1) // 2))
        nc.scalar.wait_ge(se[1], 16 * (_K // 2))

    _split_multiwait(nc)
    # populate .instr bytes for extended/pseudo Pool instructions (the
    # raw-Bass path skips Bacc's codegen pass)
    from concourse.library_overlay import lower_extended_insts

    lower_extended_insts(nc)
    return nc


def _build_in_maps(img, perm):
    import ml_dtypes

    img = np.ascontiguousarray(np.asarray(img, dtype=np.float32))
    perm = np.asarray(perm, dtype=np.int32)
    pat = _patchify(img).astype(ml_dtypes.bfloat16)  # [64, 576, 768]
    build_idx = _build_idx_dg if _MODE == "dg" else _build_idx_ind
    in_maps = []
    for c in range(_NCORES):
        sl = slice(_IMGS_PER_CORE * c, _IMGS_PER_CORE * (c + 1))
        in_maps.append(
            {
                "src": np.ascontiguousarray(pat[sl]).reshape(_N, _ELEM),
                "idx": build_idx(perm[sl]),
            }
        )
    return in_maps


def _out_to_img(out_core):
    # [128, 36*768] (out row p*36+j at [p, j*768:(j+1)*768]) -> [8,3,384,384]
    return _unpatchify(
        np.asarray(out_core).astype(np.float32).reshape(
            _IMGS_PER_CORE, _NPATCH, _ELEM
        )
    )


def _run(img, perm, trace=False):
    import sys

    if "/opt/trn_rl_repo" not in sys.path:
        sys.path.insert(0, "/opt/trn_rl_repo")
    from concourse.bass_utils import run_bass_kernel_spmd

    in_maps = _build_in_maps(img, perm)
    nc = _build_nc()
    res = run_bass_kernel_spmd(nc, in_maps, list(range(_NCORES)), trace=trace)
    out = np.concatenate([_out_to_img(r["out"]) for r in res.results], axis=0)
    return out, res


def kernel(img, perm):
    out, _ = _run(img, perm, trace=False)
    return out


# revision 8
# speedup vs baseline: 1.0343x; 1.0343x over previous
# Patch-shuffle kernel for Trainium2 (Bass), 8-way data parallel.
#
# Problem: img [64,3,384,384] f32, perm [64,576] int32 (per-image permutation
# of 16x16 patches in row-major (py,px) order). Output = per-image patch
# gather reassembled into image layout.
#
# Strategy: host repacks each image into patch-major layout [576, 768]
# (a perm-independent layout transform, part of sharding) and converts the
# payload to bf16 (halves the HBM roofline; ~2^-9 max relative rounding
# error, far inside the 2e-2 gate). The device gathers all 4608 patches of
# its 8 images from DRAM into SBUF so that partition p accumulates output
# rows [36p, 36p+36) in order, making every store back to DRAM a fully
# contiguous per-partition HWDGE DMA. Stores alternate on the Sync/Act
# HWDGE queues and overlap subsequent gathers; nothing shares SBUF
# buffers, so the whole pipeline runs without serialization.
#
# Two gather implementations:
#   "dg"  -- chunked InstDMAGatherAnt (Q7 ucode, mlp library): one
#            instruction gathers 512 patches (~1.2us issue), so the SWDGE
#            issue rate never binds.
#   "ind" -- one SWDGE indirect_dma_start per output patch column (36 x
#            ~1.4us issue; HW consumes ONE index per partition per
#            instruction). Fallback if the ucode library path misbehaves.
import numpy as np

_MODE = "dg"

_NCORES = 8
_IMGS_PER_CORE = 8
_NPATCH = 576  # 24*24 patches per image
_ELEM = 768  # payload elements per patch (3*16*16)
_N = _NPATCH * _IMGS_PER_CORE  # 4608 patches per core
_PPB = _N // 128  # 36 output patch rows per SBUF partition
_K = 9 if _MODE == "dg" else 6  # gather/store chunks
_CPC = _PPB // _K  # patch columns per chunk


def _patchify(img):
    # [B,3,384,384] -> [B, 576, 768] with patch o=(py*24+px), vec (c,ry,rx)
    b = img.shape[0]
    return (
        img.reshape(b, 3, 24, 16, 24, 16)
        .transpose(0, 2, 4, 1, 3, 5)
        .reshape(b, _NPATCH, _ELEM)
    )


def _unpatchify(pat):
    # [B, 576, 768] -> [B,3,384,384]
    b = pat.shape[0]
    return (
        pat.reshape(b, 24, 24, 3, 16, 16)
        .transpose(0, 3, 1, 4, 2, 5)
        .reshape(b, 3, 384, 384)
    )


def _flat_perm(perm_core):
    # perm_core: [8, 576] int32 for one core's images. Returns [128, 36]:
    # fl[p, j] = source patch row (into src [4608, ELEM]) for output patch
    # row p*36 + j.
    flat = (
        perm_core.astype(np.int64)
        + (np.arange(_IMGS_PER_CORE)[:, None] * _NPATCH)
    ).reshape(_N)
    assert flat.max() < _N
    return flat.reshape(128, _PPB)


def _build_idx_ind(perm_core):
    return np.ascontiguousarray(_flat_perm(perm_core).astype(np.int32))


def _build_idx_dg(perm_core):
    # InstDMAGatherAnt index layout, chunked: chunk c gathers items
    # i=0..CPC*128-1 with dst[i%128, i//128] -- we want dst[p, j] to be
    # output row p*36 + c*CPC + j, so item i=j*128+p carries
    # fl[p, c*CPC+j]. The int16 index tile wraps items in 16 partitions
    # (item i at [i%16, i//16]) and is replicated across the 8 gpsimd
    # cores' partition groups.
    fl = _flat_perm(perm_core)  # [128, 36]
    cols = []
    for c in range(_K):
        arr = fl[:, c * _CPC : (c + 1) * _CPC]  # [128, CPC]
        lin = arr.T.reshape(_CPC * 128)  # item i = j*128+p
        tile16 = lin.reshape(_CPC * 8, 16).T  # [16, CPC*8]
        cols.append(np.tile(tile16, (8, 1)))  # [128, CPC*8]
    return np.ascontiguousarray(np.concatenate(cols, axis=1).astype(np.int16))


def _split_multiwait(nc):
    # TRN2 allows at most one sync wait per instruction; hoist extra waits
    # onto same-engine nops placed immediately before the instruction.
    # (Safety net -- the manual-semaphore program below emits at most one
    # wait per instruction already.)
    from concourse import mybir

    eng_map = {
        mybir.EngineType.Pool: nc.gpsimd,
        mybir.EngineType.SP: nc.sync,
        mybir.EngineType.Activation: nc.scalar,
        mybir.EngineType.PE: nc.tensor,
        mybir.EngineType.DVE: nc.vector,
    }
    blocks = [b for f in nc.m.functions for b in f.blocks]
    multi = []
    for blk in blocks:
        for inst in blk.instructions:
            si = inst.sync_info
            if si and si.on_wait and len(si.on_wait) > 1:
                multi.append((blk, inst))
    for blk, inst in multi:
        eng = eng_map.get(inst.engine, nc.sync)
        waits = list(inst.sync_info.on_wait)
        helpers = []
        for w in waits[:-1]:
            nop = eng.nop().ins
            for b2 in blocks:
                if nop in b2.instructions:
                    b2.instructions.remove(nop)
                    break
            nop.sync_info = mybir.SyncInfo(on_wait=[w], on_update=[])
            helpers.append(nop)
        inst.sync_info.on_wait = [waits[-1]]
        pos = blk.instructions.index(inst)
        for j, h in enumerate(helpers):
            blk.instructions.insert(pos + j, h)


def _build_nc():
    from contextlib import ExitStack

    import concourse.bass as bass
    from concourse import mybir

    nc = bass.Bass(dynamic_dma_scratch_size=65536)
    src_ext = nc.dram_tensor(
        "src", [_N, _ELEM], mybir.dt.bfloat16, kind="ExternalInput"
    )
    if _MODE == "dg":
        idx_ext = nc.dram_tensor(
            "idx", [128, _N // 16], mybir.dt.int16, kind="ExternalInput"
        )
    else:
        idx_ext = nc.dram_tensor(
            "idx", [128, _PPB], mybir.dt.int32, kind="ExternalInput"
        )
    out_ext = nc.dram_tensor(
        "out", [128, _PPB, _ELEM], mybir.dt.bfloat16, kind="ExternalOutput"
    )

    with ExitStack() as stack:
        data = stack.enter_context(
            nc.sbuf_tensor("data", [128, _PPB, _ELEM], mybir.dt.bfloat16)
        )
        idx_tile = stack.enter_context(
            nc.sbuf_tensor("idxs", list(idx_ext.shape), idx_ext.dtype)
        )
        sio = stack.enter_context(nc.semaphore("sio"))
        gs = [
            stack.enter_context(nc.semaphore(f"gs{k}")) for k in range(_K)
        ]
        se = [stack.enter_context(nc.semaphore(f"se{i}")) for i in range(2)]

        if _MODE == "dg":
            from concourse import library_config

            nc.gpsimd.load_library(library_config.mlp)
        # idx loads via HWDGE so it runs while gpsimd sets up
        nc.sync.dma_start(idx_tile[:], idx_ext[:]).then_inc(sio, 16)
        if _MODE == "dg":
            # hoist the num_idxs register before the idx-load wait so the
            # per-chunk MOVE chain doesn't delay the first gather
            nidx = _CPC * 128
            nidx_reg = nc.gpsimd.to_reg(nidx)
        nc.gpsimd.wait_ge(sio, 16)
        if _MODE == "dg":
            for k in range(_K):
                nc.gpsimd.dma_gather(
                    data[:, k * _CPC : (k + 1) * _CPC, :],
                    src_ext[:],
                    idx_tile[:, k * _CPC * 8 : (k + 1) * _CPC * 8],
                    nidx,
                    nidx_reg,
                    _ELEM,
                ).then_inc(gs[k], 16)
            chunk_incs = 16
        else:
            # HW indirect DMA consumes ONE index per partition per
            # instruction -- one gather per output patch column. Gathers
            # for the same store chunk share a semaphore; the store waits
            # for the full chunk total, which is exact.
            for j in range(_PPB):
                nc.gpsimd.indirect_dma_start(
                    out=data[:, j, :],
                    out_offset=None,
                    in_=src_ext[:],
                    in_offset=bass.IndirectOffsetOnAxis(
                        ap=idx_tile[:, j : j + 1], axis=0
                    ),
                ).then_inc(gs[j // _CPC], 16)
            chunk_incs = 16 * _CPC
        store_engines = [nc.sync, nc.scalar]
        for k in range(_K):
            eng = store_engines[k % 2]
            cs, ce = k * _CPC, (k + 1) * _CPC
            eng.wait_ge(gs[k], chunk_incs)
            eng.dma_start(
                out_ext[:, cs:ce, :], data[:, cs:ce, :]
            ).then_inc(se[k % 2], 16)
        nc.sync.wait_ge(se[0], 16 * ((_K + 1) // 2))
        nc.scalar.wait_ge(se[1], 16 * (_K // 2))

    _split_multiwait(nc)
    # populate .instr bytes for extended/pseudo Pool instructions (the
    # raw-Bass path skips Bacc's codegen pass)
    from concourse.library_overlay import lower_extended_insts

    lower_extended_insts(nc)
    return nc


def _build_in_maps(img, perm):
    import ml_dtypes

    img = np.ascontiguousarray(np.asarray(img, dtype=np.float32))
    perm = np.asarray(perm, dtype=np.int32)
    pat = _patchify(img).astype(ml_dtypes.bfloat16)  # [64, 576, 768]
    build_idx = _build_idx_dg if _MODE == "dg" else _build_idx_ind
    in_maps = []
    for c in range(_NCORES):
        sl = slice(_IMGS_PER_CORE * c, _IMGS_PER_CORE * (c + 1))
        in_maps.append(
            {
                "src": np.ascontiguousarray(pat[sl]).reshape(_N, _ELEM),
                "idx": build_idx(perm[sl]),
            }
        )
    return in_maps


def _out_to_img(out_core):
    # [128, 36*768] (out row p*36+j at [p, j*768:(j+1)*768]) -> [8,3,384,384]
    return _unpatchify(
        np.asarray(out_core).astype(np.float32).reshape(
            _IMGS_PER_CORE, _NPATCH, _ELEM
        )
    )


def _run(img, perm, trace=False):
    import sys

    if "/opt/trn_rl_repo" not in sys.path:
        sys.path.insert(0, "/opt/trn_rl_repo")
    from concourse.bass_utils import run_bass_kernel_spmd

    in_maps = _build_in_maps(img, perm)
    nc = _build_nc()
    res = run_bass_kernel_spmd(nc, in_maps, list(range(_NCORES)), trace=trace)
    out = np.concatenate([_out_to_img(r["out"]) for r in res.results], axis=0)
    return out, res


def kernel(img, perm):
    out, _ = _run(img, perm, trace=False)
    return out
